# revision 1
# baseline (speedup 1.0000x reference)
"""Multi-head attention (B=2, S=2048, E=768, H=12, D=64) on 8 NeuronCores.

Sharding: core c -> batch b = c//4, head group hg = c%4 (3 heads each).
Each core computes the qkv projection for its 3 heads, attention, and a
partial output projection (rows of w_proj for its heads). Host sums the
partials per batch and adds the bias terms (tensor-parallel unshard).

Device dataflow (everything transposed so no on-chip transposes are needed,
and every matmul has a full K=128 contraction — K=64 matmuls run the PE at
half clock):
  xT [768, 2048]  (host-pretransposed, fp16), loaded in column halves so
           the projection's partial-K PSUM groups start during the DMA
           stream (no warmup spin needed).
  qkT[t] = (w_qk_tile_t)^T @ xT -> [128, 2048] tiles t=0..4 with w columns
           laid out [q0|q1], [k0|0], [0|k1], [0|q2], [0|k2]: each head's
           scoresT matmul then uses a full-128-partition stationary whose
           zero half kills the other head's rows.
  v'   = xT_tile^T @ w_v -> per-head per-Sk-block [128, 128] blocks:
           cols 0:64 = 1.0, cols 64:128 = v, so the AV matmul produces the
           softmax denominator in PSUM rows 0:64 (partition base 0, where
           the DVE reciprocal can read it directly) and values in 64:128.
  pT   = exp(scoresT / 8)   (ScalarE, PSUM -> SBUF fp16)
  avT  = v'^T @ pT          -> [128, 512] PSUM per (head, Sq-chunk), with
           the matmul stream lag-emitted behind the exp stream so each
           head's AV fills the next head's scores window.
  outT = av[64:128] * approx(1/av[0:64])  (DVE, misaligned-partition mul)
  yA   = w_proj[heads01]^T @ st01   emitted mid-kernel once heads 0/1 are
           normalized; its own DRAM buffer, summed on the host.
  yB   = w_proj[head2]^T @ outT2    in the tail. Head-2 AV accumulates
           sk 0..14 into a staged SBUF partial so only one matmul + a DVE
           add sit behind the final exp.
"""

import threading

import numpy as np

import concourse.bass as bass
import concourse.tile as tile
from concourse import bacc, mybir
from concourse.bass import ts, ds
from concourse.bass_utils import run_bass_kernel_spmd

F32 = mybir.dt.float32
F16 = mybir.dt.float16

EMBED = 768
NH = 12
D = 64
B = 2
S = 2048
HPC = 3          # heads per core
NCORES = 8
P = 128
KC = EMBED // P  # 6 contraction chunks
NQ = S // 512    # 4 Sq chunks of 512
NSK = S // P     # 16 Sk blocks
NT = 5           # qk projection tiles

DEBUG_DUMP = False  # add qkT/vp/st01/outT2 DRAM dumps for stage bisection


def _build_kernel(nc):
    xT = nc.dram_tensor("xT", [EMBED, S], F16, kind="ExternalInput").ap()
    wqk = nc.dram_tensor("w_qk", [EMBED, 3 * P], F16, kind="ExternalInput").ap()
    b2d = nc.dram_tensor("b2", [P, 2], F32, kind="ExternalInput").ap()
    wv = nc.dram_tensor("w_v", [EMBED, HPC * D], F16, kind="ExternalInput").ap()
    wp = nc.dram_tensor("w_p", [2 * P, EMBED], F16, kind="ExternalInput").ap()
    yA = nc.dram_tensor("yA", [EMBED, S], F16, kind="ExternalOutput").ap()
    yB = nc.dram_tensor("yB", [EMBED, S], F16, kind="ExternalOutput").ap()

    with tile.TileContext(nc) as tc:
        with (
            tc.tile_pool(name="wpool", bufs=1) as wpool,
            tc.tile_pool(name="xpool", bufs=1) as xpool,
            tc.tile_pool(name="qkpool", bufs=1) as qkpool,
            tc.tile_pool(name="vpool", bufs=1) as vpool,
            tc.tile_pool(name="ptpool", bufs=24) as ptpool,
            tc.tile_pool(name="opool", bufs=1) as opool,
            tc.tile_pool(name="rlpool", bufs=4) as rlpool,
            tc.tile_pool(name="psum", bufs=3, space="PSUM") as psum,
        ):
            # ---- loads: wqk_k + xT_k column-half-0 interleaved, then the
            # column-half-1s, so the early partial-K groups for Sq 0:1024
            # finish before the full xT is resident.
            wqk_t = []
            xT_t = []
            for k in range(KC):
                wqk_k = wpool.tile([P, 3 * P], F16, name=f"wqk{k}")
                nc.sync.dma_start(out=wqk_k, in_=wqk[ts(k, P), :])
                wqk_t.append(wqk_k)
                xT_k = xpool.tile([P, S], F16, name=f"xT{k}")
                nc.sync.dma_start(out=xT_k, in_=xT[ts(k, P), :])
                xT_t.append(xT_k)
            b2 = wpool.tile([P, 2], F32, name="b2")
            nc.sync.dma_start(out=b2, in_=b2d)
            wv_t = []
            for k in range(KC):
                wv_k = wpool.tile([P, HPC * D], F16, name=f"wv{k}")
                nc.sync.dma_start(out=wv_k, in_=wv[ts(k, P), :])
                wv_t.append(wv_k)
            wpA = wpool.tile([P, EMBED], F16)
            nc.sync.dma_start(out=wpA, in_=wp[0:P, :])
            wpB = wpool.tile([P, EMBED], F16)
            nc.sync.dma_start(out=wpB, in_=wp[P : 2 * P, :])

            # ---- persistent sbuf tensors + memsets (gpsimd, idle early)
            qkT = {
                t: qkpool.tile([P, S], F16, name=f"qkT{t}", tag=f"qkT{t}")
                for t in range(NT)
            }
            for t in (1, 2, 3, 4):
                zero = slice(D, P) if t == 1 else slice(0, D)
                nc.gpsimd.memset(qkT[t][zero, :], 0.0)
            vp = []
            for h in range(HPC):
                vp_h = vpool.tile([P, NSK * P], F16, name=f"vp{h}", tag=f"vp{h}")
                nc.gpsimd.memset(
                    vp_h.rearrange("p (s c) -> p s c", c=P)[:, :, 0:D], 1.0
                )
                vp.append(vp_h)
            st01 = opool.tile([P, S], F16)   # heads 0 (rows 0:64) and 1 (64:128)
            outT2 = opool.tile([P, S], F16)  # head 2 (rows 0:64; 64:128 zeroed)
            nc.gpsimd.memset(outT2[D:P, :], 0.0)

            # warm the Exp table while the loads stream
            wexp = rlpool.tile([P, 1], F32, name="wexp", tag="wexp", bufs=1)
            nc.scalar.activation(
                out=wexp, in_=wqk_t[0][:, 0:1],
                func=mybir.ActivationFunctionType.Exp, scale=1.0
            )

            # ---- early qk projection: 8 partial-K PSUM groups fed as the
            # xT tiles land. sc slots hold [t0n0|t0n1], [t0n2|t0n3],
            # [t1n0|t1n1]; av slots hold t1n2, t1n3.
            scA = psum.tile([P, 1024], F32, tag="sc", name="scA")
            scB = psum.tile([P, 1024], F32, tag="sc", name="scB")
            scC = psum.tile([P, 1024], F32, tag="sc", name="scC")
            avA = psum.tile([P, 512], F32, tag="av", bufs=2, name="avA")
            avB = psum.tile([P, 512], F32, tag="av", bufs=2, name="avB")
            egroups = [
                (scA[:, 0:512], 0, 0),
                (scA[:, 512:1024], 0, 1),
                (scC[:, 0:512], 1, 0),
                (scC[:, 512:1024], 1, 1),
                (scB[:, 0:512], 0, 2),
                (scB[:, 512:1024], 0, 3),
                (avA, 1, 2),
                (avB, 1, 3),
            ]
            for k in range(KC):
                for dst, t, nq in egroups:
                    nc.tensor.matmul(
                        dst,
                        lhsT=wqk_t[k][:, ts(t, P)],
                        rhs=xT_t[k][:, ts(nq, 512)],
                        start=(k == 0),
                        stop=(k == KC - 1),
                    )
            # drains (vector): earliest-scores needs qkT0 fully + qkT1 cols 0:1024
            nc.scalar.activation(
                out=qkT[0][:, 0:1024], in_=scA,
                func=mybir.ActivationFunctionType.Identity, bias=b2[:, 0:1],
            )
            nc.vector.tensor_copy(out=qkT[1][0:D, 0:1024], in_=scC[0:D, :])
            nc.scalar.activation(
                out=qkT[0][:, 1024:2048], in_=scB,
                func=mybir.ActivationFunctionType.Identity, bias=b2[:, 0:1],
            )
            nc.vector.tensor_copy(out=qkT[1][0:D, ts(2, 512)], in_=avA[0:D, :])
            nc.vector.tensor_copy(out=qkT[1][0:D, ts(3, 512)], in_=avB[0:D, :])
            nc.vector.tensor_copy(out=qkT[2][D:P, 0:1024], in_=scC[D:P, :])
            nc.vector.tensor_copy(out=qkT[2][D:P, ts(2, 512)], in_=avA[D:P, :])
            nc.vector.tensor_copy(out=qkT[2][D:P, ts(3, 512)], in_=avB[D:P, :])

            # ---- kernel building blocks
            def scores_step(kt, qt, pts):
                sk = len(pts)
                pt = ptpool.tile([P, S], F16, name="pt", tag="pt")
                pts.append(pt)
                for g in range(2):
                    sps = psum.tile([P, 1024], F32, tag="sc", name="ps_s")
                    for j in range(2):
                        nc.tensor.matmul(
                            sps[:, ts(j, 512)],
                            lhsT=kt[:, ts(sk, P)],
                            rhs=qt[:, ds(g * 1024 + j * 512, 512)],
                            start=True,
                            stop=True,
                        )
                    nc.scalar.activation(
                        out=pt[:, ts(g, 1024)],
                        in_=sps,
                        func=mybir.ActivationFunctionType.Exp,
                        scale=float(D) ** -0.5,
                    )

            def qk2_chunk(n01):
                """tile2 = [q2|k2] projection for Sq chunks n01, n01+1:
                q2 -> qkT3 rows 64:128, k2 -> qkT4 rows 64:128."""
                sc = psum.tile([P, 1024], F32, tag="sc", name="ps_qk2")
                for k in range(KC):
                    for j in range(2):
                        nc.tensor.matmul(
                            sc[:, ts(j, 512)],
                            lhsT=wqk_t[k][:, ts(2, P)],
                            rhs=xT_t[k][:, ts(n01 + j, 512)],
                            start=(k == 0),
                            stop=(k == KC - 1),
                        )
                nc.vector.tensor_scalar_add(
                    out=qkT[3][D:P, ds(n01 * 512, 1024)],
                    in0=sc[0:D, :],
                    scalar1=b2[D:P, 1:2],
                )
                nc.vector.tensor_copy(
                    out=qkT[4][D:P, ds(n01 * 512, 1024)], in_=sc[D:P, :]
                )

            def v_super(g4):
                """v projection for Sk blocks 4*g4 .. 4*g4+3 in one sc slot;
                one strided drain copy per head."""
                # 256-col pitch per group: a 192-wide accumulation group must
                # not straddle a 512-float PSUM bank boundary
                vps = psum.tile([P, 1024], F32, tag="sc", name="ps_v")
                for j in range(4):
                    st = 4 * g4 + j
                    for k in range(KC):
                        nc.tensor.matmul(
                            vps[:, ds(j * 256, HPC * D)],
                            lhsT=xT_t[k][:, ts(st, P)],
                            rhs=wv_t[k],
                            start=(k == 0),
                            stop=(k == KC - 1),
                        )
                vv = vps.rearrange("p (j r) -> p j r", r=256)
                for h in range(HPC):
                    nc.vector.tensor_copy(
                        out=vp[h].rearrange("p (s c) -> p s c", c=P)[
                            :, ds(4 * g4, 4), D:P
                        ],
                        in_=vv[:, :, ds(h * D, D)],
                    )

            def norm_chunk(h, nq, avp):
                rr = rlpool.tile([D, 512], F32, name="rr", tag="rr")
                nc.vector.reciprocal_approx_fast(out=rr, in_=avp[0:D, :])
                if h == 0:
                    dst = st01[0:D, ts(nq, 512)]
                elif h == 1:
                    dst = st01[D:P, ts(nq, 512)]
                else:
                    dst = outT2[0:D, ts(nq, 512)]
                nc.vector.tensor_mul(out=dst, in0=avp[D:P, :], in1=rr)

            class AvChunk:
                """Lag-emitted AV accumulation: emit() is called once per
                scores step so the matmul stream trails the exp stream."""

                def __init__(self, h, pts, nq, nsk, finish):
                    self.h, self.pts, self.nq, self.nsk, self.finish = (
                        h, pts, nq, nsk, finish)
                    self.av = psum.tile([P, 512], F32, tag="av", bufs=2, name="ps_av")
                    self.j = 0

                def emit(self, upto, cap=NSK):
                    upto = min(upto, self.j + cap)
                    while self.j < min(upto, self.nsk):
                        j = self.j
                        nc.tensor.matmul(
                            self.av,
                            lhsT=vp[self.h][:, ts(j, P)],
                            rhs=self.pts[j][:, ts(self.nq, 512)],
                            start=(j == 0),
                            stop=(j == self.nsk - 1),
                        )
                        self.j += 1
                    if self.j == self.nsk:
                        self.j += 1
                        self.finish(self.av)

            def av_burst(h, pts, nq):
                c = AvChunk(h, pts, nq, NSK, lambda av: norm_chunk(h, nq, av))
                c.emit(NSK)

            a2sb = {}

            def av2a_stage(nq, av):
                sb = rlpool.tile([P, 512], F32, name="a2s", tag="a2s", bufs=4)
                nc.vector.tensor_copy(out=sb, in_=av)
                a2sb[nq] = sb

            def y_chunk(wt, src, ydram, nq, engines, dma_engines=("y",)):
                for mt2 in range(3):
                    yps = psum.tile([P, 1024], F32, tag="sc", name="ps_y")
                    for j in range(2):
                        nc.tensor.matmul(
                            yps[:, ts(j, 512)],
                            lhsT=wt[:, ts(2 * mt2 + j, P)],
                            rhs=src[:, ts(nq, 512)],
                            start=True,
                            stop=True,
                        )
                    ysb = rlpool.tile([P, 1024], F16, name="ysb", tag="ysb", bufs=4)
                    if engines[mt2 % len(engines)] == "s":
                        nc.scalar.copy(out=ysb, in_=yps)
                    else:
                        nc.vector.tensor_copy(out=ysb, in_=yps)
                    # one DMA per staged tile: dram viewed [p, two, c] so the
                    # partition dim leads on both sides
                    eng = dma_engines[mt2 % len(dma_engines)]
                    (nc.scalar if eng == "s" else nc.sync).dma_start(
                        out=ydram[ds(2 * mt2 * P, 2 * P), ts(nq, 512)].rearrange(
                            "(two p) c -> p two c", p=P
                        ),
                        in_=ysb.rearrange("p (two c) -> p two c", c=512),
                    )

            # ================= emission schedule =================
            pts0, pts1, pts2 = [], [], []

            # ---- h0 scores; fillers: remaining qk/v chunks + lagged av0
            lagged = []
            fill0 = {
                1: lambda: v_super(0),
                3: lambda: qk2_chunk(0),
                5: lambda: v_super(1),
                7: lambda: qk2_chunk(2),
                9: lambda: v_super(2),
                11: lambda: v_super(3),
            }
            for sk in range(NSK):
                scores_step(qkT[1], qkT[0], pts0)
                if sk in fill0:
                    fill0[sk]()
                if sk == 2:
                    lagged.append(AvChunk(0, pts0, 0, NSK,
                                          lambda av: norm_chunk(0, 0, av)))
                if sk == 4:
                    lagged.append(AvChunk(0, pts0, 1, NSK,
                                          lambda av: norm_chunk(0, 1, av)))
                for c in lagged:
                    c.emit(sk - 1)

            # ---- h1 scores; fillers: av0 tail bursts + lagged av1
            fast = []
            for sk in range(NSK):
                scores_step(qkT[2], qkT[0], pts1)
                if sk == 0:
                    for c in lagged:
                        c.emit(NSK)
                    lagged = []
                    fast.append(AvChunk(0, pts0, 2, NSK,
                                        lambda av: norm_chunk(0, 2, av)))
                if sk == 4:
                    fast.append(AvChunk(0, pts0, 3, NSK,
                                        lambda av: norm_chunk(0, 3, av)))
                for c in fast:
                    c.emit(NSK, cap=4)
                if sk == 2:
                    lagged.append(AvChunk(1, pts1, 0, NSK,
                                          lambda av: norm_chunk(1, 0, av)))
                if sk == 4:
                    lagged.append(AvChunk(1, pts1, 1, NSK,
                                          lambda av: norm_chunk(1, 1, av)))
                for c in lagged:
                    c.emit(sk - 1)

            # ---- h2 scores; fillers: av1 tail bursts, lagged av2a, yA
            fast = []
            for sk in range(NSK):
                scores_step(qkT[4], qkT[3], pts2)
                if sk == 0:
                    for c in lagged:
                        c.emit(NSK)
                    lagged = []
                    fast.append(AvChunk(1, pts1, 2, NSK,
                                        lambda av: norm_chunk(1, 2, av)))
                if sk == 4:
                    fast.append(AvChunk(1, pts1, 3, NSK,
                                        lambda av: norm_chunk(1, 3, av)))
                for c in fast:
                    c.emit(NSK, cap=4)
                if sk == 2:
                    lagged.append(AvChunk(2, pts2, 0, NSK - 1,
                                          lambda av: av2a_stage(0, av)))
                if sk == 4:
                    lagged.append(AvChunk(2, pts2, 1, NSK - 1,
                                          lambda av: av2a_stage(1, av)))
                if sk in (2, 5, 8):
                    y_chunk(wpA, st01, yA, (sk - 2) // 3, engines=("v",))
                if sk == 13:
                    y_chunk(wpA, st01, yA, 3, engines=("v",))
                for c in lagged:
                    c.emit(min(sk - 1, NSK - 3))
            for c in lagged:
                c.emit(NSK - 1)

            # ---- tail: av2b for n0/n1 (one matmul past the staged sk0..14
            # partial), full AV chunks for n2/n3 woven between the yB
            # projections so the PE stays busy while the DVE normalizes
            def av2b_norm(nq):
                av2b = psum.tile([P, 512], F32, tag="av", bufs=2, name="ps_a2b")
                nc.tensor.matmul(
                    av2b,
                    lhsT=vp[2][:, ts(NSK - 1, P)],
                    rhs=pts2[NSK - 1][:, ts(nq, 512)],
                    start=True,
                    stop=True,
                )
                # half-adds keep every operand pair partition-aligned (the
                # BIR verifier rejects SBUF+SBUF inputs at different bases)
                s2l = rlpool.tile([D, 512], F32, name="s2l", tag="s2l", bufs=2)
                nc.vector.tensor_add(out=s2l, in0=av2b[0:D, :], in1=a2sb[nq][0:D, :])
                s2v = rlpool.tile([D, 512], F32, name="s2v", tag="s2v", bufs=2)
                nc.vector.tensor_add(out=s2v, in0=av2b[D:P, :], in1=a2sb[nq][D:P, :])
                rr2 = rlpool.tile([D, 512], F32, name="rr2", tag="rr")
                nc.vector.reciprocal_approx_fast(out=rr2, in_=s2l)
                nc.vector.tensor_mul(out=outT2[0:D, ts(nq, 512)], in0=s2v, in1=rr2)

            av2b_norm(0)
            av2b_norm(1)
            av_burst(2, pts2, 2)
            y_chunk(wpB, outT2, yB, 0, engines=("s", "v"))
            y_chunk(wpB, outT2, yB, 1, engines=("s", "v"))
            av_burst(2, pts2, 3)
            y_chunk(wpB, outT2, yB, 2, engines=("s", "v"))
            y_chunk(wpB, outT2, yB, 3, engines=("s", "v"))

            if DEBUG_DUMP:
                for t in range(NT):
                    dbg = nc.dram_tensor(f"dbg_qkT{t}", [P, S], F16,
                                         kind="ExternalOutput").ap()
                    nc.sync.dma_start(out=dbg, in_=qkT[t])
                for h in range(HPC):
                    dbg = nc.dram_tensor(f"dbg_vp{h}", [P, NSK * P], F16,
                                         kind="ExternalOutput").ap()
                    nc.sync.dma_start(out=dbg, in_=vp[h])
                dbg = nc.dram_tensor("dbg_st01", [P, S], F16,
                                     kind="ExternalOutput").ap()
                nc.sync.dma_start(out=dbg, in_=st01)
                dbg = nc.dram_tensor("dbg_outT2", [P, S], F16,
                                     kind="ExternalOutput").ap()
                nc.sync.dma_start(out=dbg, in_=outT2)
                dbg = nc.dram_tensor("dbg_pt0", [P, S], F16,
                                     kind="ExternalOutput").ap()
                nc.sync.dma_start(out=dbg, in_=pts0[0])
                dbg = nc.dram_tensor("dbg_pt2_15", [P, S], F16,
                                     kind="ExternalOutput").ap()
                nc.sync.dma_start(out=dbg, in_=pts2[NSK - 1])
                for nq in range(NQ):
                    dbg = nc.dram_tensor(f"dbg_a2s{nq}", [P, 512], F32,
                                         kind="ExternalOutput").ap()
                    nc.sync.dma_start(out=dbg, in_=a2sb[nq])
    return nc


_CACHE = threading.Lock(), {}


def _get_nc():
    lock, cache = _CACHE
    with lock:
        if "nc" not in cache:
            nc = bacc.Bacc("TRN2", target_bir_lowering=False, debug=False)
            _build_kernel(nc)
            nc.compile()
            cache["nc"] = nc
        return cache["nc"]


def _shard_inputs(x, w_qkv, b_qkv, w_proj):
    """Build the 8 per-core input maps (host-side sharding/layout)."""
    in_maps = []
    for c in range(NCORES):
        b = c // 4
        hg = c % 4
        h0 = HPC * hg
        qc = [np.s_[D * (h0 + i) : D * (h0 + i + 1)] for i in range(HPC)]
        kc = [np.s_[EMBED + D * (h0 + i) : EMBED + D * (h0 + i + 1)] for i in range(HPC)]
        vc = [np.s_[2 * EMBED + D * (h0 + i) : 2 * EMBED + D * (h0 + i + 1)] for i in range(HPC)]

        # projected w tiles: [q0|q1], [k0|k1], [q2|k2]
        w_qk = np.zeros((EMBED, 3 * P), dtype=np.float32)
        halves = [
            (0, 0, qc[0]), (0, 1, qc[1]),
            (1, 0, kc[0]), (1, 1, kc[1]),
            (2, 0, qc[2]), (2, 1, kc[2]),
        ]
        for t, half, cols in halves:
            w_qk[:, t * P + half * D : t * P + half * D + D] = w_qkv[:, cols]
        # q biases only: col 0 = [q0|q1], col 1 rows 64:128 = q2. The k
        # biases are constant over keys and cancel in the softmax; the v
        # bias is applied on the host.
        b2 = np.zeros((P, 2), dtype=np.float32)
        b2[0:D, 0] = b_qkv[qc[0]]
        b2[D:P, 0] = b_qkv[qc[1]]
        b2[D:P, 1] = b_qkv[qc[2]]

        w_v = np.concatenate([w_qkv[:, s] for s in vc], axis=1)
        # w_proj rows for these heads; B half zero-padded to K=128
        w_p = np.zeros((2 * P, EMBED), dtype=np.float32)
        w_p[0:P] = w_proj[D * h0 : D * h0 + P]
        w_p[P : P + D] = w_proj[D * h0 + P : D * (h0 + HPC)]
        in_maps.append(
            {
                "xT": np.ascontiguousarray(x[b].T).astype(np.float16),
                "w_qk": w_qk.astype(np.float16),
                "b2": b2,
                "w_v": np.ascontiguousarray(w_v).astype(np.float16),
                "w_p": w_p.astype(np.float16),
            }
        )
    return in_maps


def kernel(x, w_qkv, b_qkv, w_proj, b_proj, _results_hook=None):
    x = np.asarray(x, dtype=np.float32)
    w_qkv = np.asarray(w_qkv, dtype=np.float32)
    b_qkv = np.asarray(b_qkv, dtype=np.float32)
    w_proj = np.asarray(w_proj, dtype=np.float32)
    b_proj = np.asarray(b_proj, dtype=np.float32)

    nc = _get_nc()
    in_maps = _shard_inputs(x, w_qkv, b_qkv, w_proj)
    res = run_bass_kernel_spmd(nc, in_maps, core_ids=list(range(NCORES)))
    if _results_hook is not None:
        _results_hook(res)

    # unshard: sum the 4 head-group partials per batch, add bias terms
    b_v = b_qkv[2 * EMBED :]
    bias_row = b_v @ w_proj + b_proj  # [768]
    out = np.empty((B, S, EMBED), dtype=np.float32)
    for b in range(B):
        acc = np.zeros((EMBED, S), dtype=np.float32)
        for hg in range(4):
            r = res.results[4 * b + hg]
            acc += r["yA"].astype(np.float32)
            acc += r["yB"].astype(np.float32)
        out[b] = acc.T + bias_row
    return out



# revision 2
# speedup vs baseline: 1.0413x; 1.0413x over previous
"""Multi-head attention (B=2, S=2048, E=768, H=12, D=64) on 8 NeuronCores.

Sharding: core c -> batch b = c//4, head group hg = c%4 (3 heads each).
Each core computes the qkv projection for its 3 heads, attention, and a
partial output projection (rows of w_proj for its heads). Host sums the
partials per batch and adds the bias terms (tensor-parallel unshard).

Device dataflow (everything transposed so no on-chip transposes are needed,
and every matmul has a full K=128 contraction — K=64 matmuls run the PE at
half clock):
  xT [768, 2048]  (host-pretransposed, fp16), loaded in column halves so
           the projection's partial-K PSUM groups start during the DMA
           stream; ~30 warmup matmuls on a dummy tile keep the PE clock
           ramping while the DMA streams.
  qkT[t] = (w_qk_tile_t)^T @ xT -> [128, 2048] tiles t=0..4 with w columns
           laid out [q0|q1], [k0|0], [0|k1], [0|q2], [0|k2]: each head's
           scoresT matmul then uses a full-128-partition stationary whose
           zero half kills the other head's rows.  The q columns of w_qkv
           (and q biases) are host-prescaled by 1024*log2(e)/8 so the
           scores PSUM is directly in fp16-Schraudolph J units.
  v'   = xT_tile^T @ w_v -> per-head per-Sk-block [128, 128] blocks:
           cols 0:64 = 1.0, cols 64:128 = v, so the AV matmul produces the
           softmax denominator in PSUM rows 0:64 and values in 64:128.
  pT   = exp(scores) computed split across two engines per scores step:
           g=0 -> ScalarE table exp (scale=1/(1024*log2 e)), fp16 out;
           g=1 -> VectorE one-op Schraudolph: int16(psum + (15360+delta)),
           written through a bitcast AP into the same fp16 pt tile.  Both
           produce ~e^s; numerator and denominator use the same values so
           the approximation error largely cancels in the softmax ratio.
  avT  = v'^T @ pT          -> [128, 512] PSUM per (head, Sq-chunk), with
           the matmul stream lag-emitted behind the exp stream.
  outT = av[64:128] * approx(1/av[0:64])  (DVE)
  y    = w_projA^T @ st01 + w_projB^T @ outT2 fused into ONE PSUM
           accumulation group per output chunk (single DRAM output, half
           the store traffic of separate yA/yB).  All y work sits in the
           tail; h2's AV chunks nq0/1 are staged through SBUF partials so
           only one matmul + adds + normalize separate the final exp from
           the y projections.
"""

import threading

import numpy as np

import concourse.bass as bass
import concourse.tile as tile
from concourse import bacc, mybir
from concourse.bass import ts, ds
from concourse.bass_utils import run_bass_kernel_spmd

F32 = mybir.dt.float32
F16 = mybir.dt.float16
I16 = mybir.dt.int16

EMBED = 768
NH = 12
D = 64
B = 2
S = 2048
HPC = 3          # heads per core
NCORES = 8
P = 128
KC = EMBED // P  # 6 contraction chunks
NQ = S // 512    # 4 Sq chunks of 512
NSK = S // P     # 16 Sk blocks
NT = 5           # qk projection tiles

# fp16 Schraudolph exp: J = s_true * 1024*log2(e) + (15360 + DELTA);
# bitcast(int16(J)) ~= e^s.  The 1024*log2(e)/8 factor is folded into the
# host-side q weights/biases so the scores PSUM is already in J units.
JSCALE = 1024.0 * np.log2(np.e)          # 1477.3195
JBIAS = 15360.0 - 45.0                   # delta=-45 balances the sawtooth
ACT_SCALE = float(1.0 / JSCALE)          # ScalarE exp reads the same psum

N_WARMUP = 26    # dummy matmuls that ramp the PE clock during the DMA-in


def _build_kernel(nc):
    xT = nc.dram_tensor("xT", [EMBED, S], F16, kind="ExternalInput").ap()
    wqk = nc.dram_tensor("w_qk", [EMBED, 3 * P], F16, kind="ExternalInput").ap()
    b2d = nc.dram_tensor("b2", [P, 2], F32, kind="ExternalInput").ap()
    wv = nc.dram_tensor("w_v", [EMBED, HPC * D], F16, kind="ExternalInput").ap()
    wp = nc.dram_tensor("w_p", [2 * P, EMBED], F16, kind="ExternalInput").ap()
    y = nc.dram_tensor("y", [EMBED, S], F16, kind="ExternalOutput").ap()

    with tile.TileContext(nc) as tc:
        with (
            tc.tile_pool(name="wpool", bufs=1) as wpool,
            tc.tile_pool(name="xpool", bufs=1) as xpool,
            tc.tile_pool(name="qkpool", bufs=1) as qkpool,
            tc.tile_pool(name="vpool", bufs=1) as vpool,
            tc.tile_pool(name="ptpool", bufs=24) as ptpool,
            tc.tile_pool(name="opool", bufs=1) as opool,
            tc.tile_pool(name="rlpool", bufs=4) as rlpool,
            tc.tile_pool(name="psum", bufs=3, space="PSUM") as psum,
        ):
            # ---- PE warmup: dummy matmuls on a memset tile keep the PE
            # busy (and its clock ramping) from t~0 while the DMAs stream.
            dummy = wpool.tile([P, 512], F16, name="dummy")
            nc.gpsimd.memset(dummy, 0.0)
            wps = psum.tile([P, 512], F32, tag="av", bufs=2, name="ps_warm")
            for _ in range(N_WARMUP):
                nc.tensor.matmul(wps, lhsT=dummy[:, 0:P], rhs=dummy,
                                 start=True, stop=True)

            # ---- loads: wqk_k + xT_k column-half-0 interleaved, then the
            # column-half-1s, so the left-half partial-K PSUM groups finish
            # before the full xT is resident.
            wqk_t = []
            xT_t = []
            for k in range(KC):
                wqk_k = wpool.tile([P, 3 * P], F16, name=f"wqk{k}")
                nc.sync.dma_start(out=wqk_k, in_=wqk[ts(k, P), :])
                wqk_t.append(wqk_k)
                xT_k = xpool.tile([P, S], F16, name=f"xT{k}")
                nc.sync.dma_start(out=xT_k[:, 0:1024], in_=xT[ts(k, P), 0:1024])
                xT_t.append(xT_k)
            b2 = wpool.tile([P, 2], F32, name="b2")
            nc.sync.dma_start(out=b2, in_=b2d)
            for k in range(KC):
                nc.sync.dma_start(
                    out=xT_t[k][:, 1024:2048], in_=xT[ts(k, P), 1024:2048]
                )
            wv_t = []
            for k in range(KC):
                wv_k = wpool.tile([P, HPC * D], F16, name=f"wv{k}")
                nc.sync.dma_start(out=wv_k, in_=wv[ts(k, P), :])
                wv_t.append(wv_k)
            wpA = wpool.tile([P, EMBED], F16)
            nc.sync.dma_start(out=wpA, in_=wp[0:P, :])
            wpB = wpool.tile([P, EMBED], F16)
            nc.sync.dma_start(out=wpB, in_=wp[P : 2 * P, :])

            # ---- persistent sbuf tensors + memsets (gpsimd, idle early)
            qkT = {
                t: qkpool.tile([P, S], F16, name=f"qkT{t}", tag=f"qkT{t}")
                for t in range(NT)
            }
            for t in (1, 2, 3, 4):
                zero = slice(D, P) if t == 1 else slice(0, D)
                nc.gpsimd.memset(qkT[t][zero, :], 0.0)
            vp = []
            for h in range(HPC):
                vp_h = vpool.tile([P, NSK * P], F16, name=f"vp{h}", tag=f"vp{h}")
                nc.gpsimd.memset(
                    vp_h.rearrange("p (s c) -> p s c", c=P)[:, :, 0:D], 1.0
                )
                vp.append(vp_h)
            st01 = opool.tile([P, S], F16)   # heads 0 (rows 0:64) and 1 (64:128)
            outT2 = opool.tile([P, S], F16)  # head 2 (rows 0:64; 64:128 zeroed)
            nc.gpsimd.memset(outT2[D:P, :], 0.0)

            # warm the Exp table while the loads stream
            wexp = rlpool.tile([P, 1], F32, name="wexp", tag="wexp", bufs=1)
            nc.scalar.activation(
                out=wexp, in_=dummy[:, 0:1],
                func=mybir.ActivationFunctionType.Exp, scale=1.0
            )

            # ---- early qk projection: 8 partial-K PSUM groups fed as the
            # xT tiles land; left-half (Sq 0:1024) groups emitted first so
            # they only depend on the half-0 DMAs.
            scA = psum.tile([P, 1024], F32, tag="sc", name="scA")
            scB = psum.tile([P, 1024], F32, tag="sc", name="scB")
            scC = psum.tile([P, 1024], F32, tag="sc", name="scC")
            avA = psum.tile([P, 512], F32, tag="av", bufs=2, name="avA")
            avB = psum.tile([P, 512], F32, tag="av", bufs=2, name="avB")
            egroups = [
                (scA[:, 0:512], 0, 0),
                (scA[:, 512:1024], 0, 1),
                (scC[:, 0:512], 1, 0),
                (scC[:, 512:1024], 1, 1),
                (scB[:, 0:512], 0, 2),
                (scB[:, 512:1024], 0, 3),
                (avA, 1, 2),
                (avB, 1, 3),
            ]
            for k in range(KC):
                for dst, t, nq in egroups:
                    nc.tensor.matmul(
                        dst,
                        lhsT=wqk_t[k][:, ts(t, P)],
                        rhs=xT_t[k][:, ts(nq, 512)],
                        start=(k == 0),
                        stop=(k == KC - 1),
                    )
            # drains (vector): earliest-scores needs qkT0 fully + qkT1 cols 0:1024
            nc.scalar.activation(
                out=qkT[0][:, 0:1024], in_=scA,
                func=mybir.ActivationFunctionType.Identity, bias=b2[:, 0:1],
            )
            nc.vector.tensor_copy(out=qkT[1][0:D, 0:1024], in_=scC[0:D, :])
            nc.scalar.activation(
                out=qkT[0][:, 1024:2048], in_=scB,
                func=mybir.ActivationFunctionType.Identity, bias=b2[:, 0:1],
            )
            nc.vector.tensor_copy(out=qkT[1][0:D, ts(2, 512)], in_=avA[0:D, :])
            nc.vector.tensor_copy(out=qkT[1][0:D, ts(3, 512)], in_=avB[0:D, :])
            nc.vector.tensor_copy(out=qkT[2][D:P, 0:1024], in_=scC[D:P, :])
            nc.vector.tensor_copy(out=qkT[2][D:P, ts(2, 512)], in_=avA[D:P, :])
            nc.vector.tensor_copy(out=qkT[2][D:P, ts(3, 512)], in_=avB[D:P, :])

            # ---- kernel building blocks
            def scores_step(kt, qt, pts):
                """One Sk block of scores for one head: 4 matmuls + 2 exps,
                g=0 via ScalarE table exp, g=1 via VectorE Schraudolph."""
                sk = len(pts)
                pt = ptpool.tile([P, S], F16, name="pt", tag="pt")
                pts.append(pt)
                for g in range(2):
                    sps = psum.tile([P, 1024], F32, tag="sc", name="ps_s")
                    for j in range(2):
                        nc.tensor.matmul(
                            sps[:, ts(j, 512)],
                            lhsT=kt[:, ts(sk, P)],
                            rhs=qt[:, ds(g * 1024 + j * 512, 512)],
                            start=True,
                            stop=True,
                        )
                    if g == 0:
                        nc.scalar.activation(
                            out=pt[:, ts(g, 1024)],
                            in_=sps,
                            func=mybir.ActivationFunctionType.Exp,
                            scale=ACT_SCALE,
                        )
                    else:
                        nc.vector.tensor_scalar_add(
                            out=pt[:, ts(g, 1024)].bitcast(I16),
                            in0=sps,
                            scalar1=JBIAS,
                        )

            def qk2_chunk(n01):
                """tile2 = [q2|k2] projection for Sq chunks n01, n01+1:
                q2 -> qkT3 rows 64:128, k2 -> qkT4 rows 64:128."""
                sc = psum.tile([P, 1024], F32, tag="sc", name="ps_qk2")
                for k in range(KC):
                    for j in range(2):
                        nc.tensor.matmul(
                            sc[:, ts(j, 512)],
                            lhsT=wqk_t[k][:, ts(2, P)],
                            rhs=xT_t[k][:, ts(n01 + j, 512)],
                            start=(k == 0),
                            stop=(k == KC - 1),
                        )
                nc.vector.tensor_scalar_add(
                    out=qkT[3][D:P, ds(n01 * 512, 1024)],
                    in0=sc[0:D, :],
                    scalar1=b2[D:P, 1:2],
                )
                nc.vector.tensor_copy(
                    out=qkT[4][D:P, ds(n01 * 512, 1024)], in_=sc[D:P, :]
                )

            def v_super(g4):
                """v projection for Sk blocks 4*g4 .. 4*g4+3 in one sc slot;
                one strided drain copy per head (ScalarE: DVE is on exp duty)."""
                # 256-col pitch per group: a 192-wide accumulation group must
                # not straddle a 512-float PSUM bank boundary
                vps = psum.tile([P, 1024], F32, tag="sc", name="ps_v")
                for j in range(4):
                    st = 4 * g4 + j
                    for k in range(KC):
                        nc.tensor.matmul(
                            vps[:, ds(j * 256, HPC * D)],
                            lhsT=xT_t[k][:, ts(st, P)],
                            rhs=wv_t[k],
                            start=(k == 0),
                            stop=(k == KC - 1),
                        )
                vv = vps.rearrange("p (j r) -> p j r", r=256)
                for h in range(HPC):
                    nc.scalar.copy(
                        out=vp[h].rearrange("p (s c) -> p s c", c=P)[
                            :, ds(4 * g4, 4), D:P
                        ],
                        in_=vv[:, :, ds(h * D, D)],
                    )

            def norm_chunk(h, nq, avp):
                rr = rlpool.tile([D, 512], F32, name="rr", tag="rr")
                nc.vector.reciprocal_approx_fast(out=rr, in_=avp[0:D, :])
                if h == 0:
                    dst = st01[0:D, ts(nq, 512)]
                elif h == 1:
                    dst = st01[D:P, ts(nq, 512)]
                else:
                    dst = outT2[0:D, ts(nq, 512)]
                nc.vector.tensor_mul(out=dst, in0=avp[D:P, :], in1=rr)

            class AvChunk:
                """Lag-emitted AV accumulation: emit() is called once per
                scores step so the matmul stream trails the exp stream."""

                def __init__(self, h, pts, nq, nsk, finish):
                    self.h, self.pts, self.nq, self.nsk, self.finish = (
                        h, pts, nq, nsk, finish)
                    self.av = psum.tile([P, 512], F32, tag="av", bufs=2, name="ps_av")
                    self.j = 0

                def emit(self, upto, cap=NSK):
                    upto = min(upto, self.j + cap)
                    while self.j < min(upto, self.nsk):
                        j = self.j
                        nc.tensor.matmul(
                            self.av,
                            lhsT=vp[self.h][:, ts(j, P)],
                            rhs=self.pts[j][:, ts(self.nq, 512)],
                            start=(j == 0),
                            stop=(j == self.nsk - 1),
                        )
                        self.j += 1
                    if self.j == self.nsk:
                        self.j += 1
                        self.finish(self.av)

            def av_burst(h, pts, nq):
                c = AvChunk(h, pts, nq, NSK, lambda av: norm_chunk(h, nq, av))
                c.emit(NSK)

            a2sb = {}

            def av2a_stage(nq, av):
                sb = rlpool.tile([P, 512], F32, name="a2s", tag="a2s", bufs=2)
                nc.vector.tensor_copy(out=sb, in_=av)
                a2sb[nq] = sb

            def y_chunk(nq, engines=("s", "v"), dma_engines=("y", "s")):
                """Fused output projection for Sq chunk nq: for each pair of
                128-row output chunks, accumulate wpA^T@st01 and wpB^T@outT2
                into one PSUM group (K=256 over the 3 heads' dims)."""
                for mt2 in range(3):
                    yps = psum.tile([P, 1024], F32, tag="sc", name="ps_y")
                    for j in range(2):
                        nc.tensor.matmul(
                            yps[:, ts(j, 512)],
                            lhsT=wpA[:, ts(2 * mt2 + j, P)],
                            rhs=st01[:, ts(nq, 512)],
                            start=True,
                            stop=False,
                        )
                        nc.tensor.matmul(
                            yps[:, ts(j, 512)],
                            lhsT=wpB[:, ts(2 * mt2 + j, P)],
                            rhs=outT2[:, ts(nq, 512)],
                            start=False,
                            stop=True,
                        )
                    ysb = rlpool.tile([P, 1024], F16, name="ysb", tag="ysb", bufs=4)
                    if engines[mt2 % len(engines)] == "s":
                        nc.scalar.copy(out=ysb, in_=yps)
                    else:
                        nc.vector.tensor_copy(out=ysb, in_=yps)
                    # one DMA per staged tile: dram viewed [p, two, c] so the
                    # partition dim leads on both sides
                    eng = dma_engines[mt2 % len(dma_engines)]
                    (nc.scalar if eng == "s" else nc.sync).dma_start(
                        out=y[ds(2 * mt2 * P, 2 * P), ts(nq, 512)].rearrange(
                            "(two p) c -> p two c", p=P
                        ),
                        in_=ysb.rearrange("p (two c) -> p two c", c=512),
                    )

            # ================= emission schedule =================
            pts0, pts1, pts2 = [], [], []

            # ---- h0 scores; fillers: remaining qk/v chunks + lagged av0
            lagged = []
            fill0 = {
                1: lambda: v_super(0),
                3: lambda: qk2_chunk(0),
                5: lambda: v_super(1),
                7: lambda: qk2_chunk(2),
                9: lambda: v_super(2),
                11: lambda: v_super(3),
            }
            for sk in range(NSK):
                scores_step(qkT[1], qkT[0], pts0)
                if sk in fill0:
                    fill0[sk]()
                if sk == 2:
                    lagged.append(AvChunk(0, pts0, 0, NSK,
                                          lambda av: norm_chunk(0, 0, av)))
                if sk == 4:
                    lagged.append(AvChunk(0, pts0, 1, NSK,
                                          lambda av: norm_chunk(0, 1, av)))
                for c in lagged:
                    c.emit(sk - 1)

            # ---- h1 scores; fillers: av0 tail bursts + lagged av1
            fast = []
            for sk in range(NSK):
                scores_step(qkT[2], qkT[0], pts1)
                if sk == 0:
                    for c in lagged:
                        c.emit(NSK)
                    lagged = []
                    fast.append(AvChunk(0, pts0, 2, NSK,
                                        lambda av: norm_chunk(0, 2, av)))
                if sk == 4:
                    fast.append(AvChunk(0, pts0, 3, NSK,
                                        lambda av: norm_chunk(0, 3, av)))
                for c in fast:
                    c.emit(NSK, cap=4)
                if sk == 2:
                    lagged.append(AvChunk(1, pts1, 0, NSK,
                                          lambda av: norm_chunk(1, 0, av)))
                if sk == 4:
                    lagged.append(AvChunk(1, pts1, 1, NSK,
                                          lambda av: norm_chunk(1, 1, av)))
                for c in lagged:
                    c.emit(sk - 1)

            # ---- h2 scores; fillers: av1 tail bursts, staged av2 (nq0/1
            # through SBUF partials) and natural-lag av2 chunks (nq2/3)
            fast = []
            av23 = []
            for sk in range(NSK):
                scores_step(qkT[4], qkT[3], pts2)
                if sk == 0:
                    for c in lagged:
                        c.emit(NSK)
                    lagged = []
                    fast.append(AvChunk(1, pts1, 2, NSK,
                                        lambda av: norm_chunk(1, 2, av)))
                if sk == 4:
                    fast.append(AvChunk(1, pts1, 3, NSK,
                                        lambda av: norm_chunk(1, 3, av)))
                for c in fast:
                    c.emit(NSK, cap=4)
                if sk == 2:
                    lagged.append(AvChunk(2, pts2, 0, NSK - 1,
                                          lambda av: av2a_stage(0, av)))
                if sk == 4:
                    lagged.append(AvChunk(2, pts2, 1, NSK - 1,
                                          lambda av: av2a_stage(1, av)))
                if sk == 8:
                    av23.append(AvChunk(2, pts2, 2, NSK,
                                        lambda av: norm_chunk(2, 2, av)))
                if sk == 10:
                    av23.append(AvChunk(2, pts2, 3, NSK,
                                        lambda av: norm_chunk(2, 3, av)))
                for c in lagged:
                    c.emit(min(sk - 1, NSK - 3))
                for c in av23:
                    c.emit(min(sk - 1, NSK - 2), cap=4)
            for c in lagged:
                c.emit(NSK - 1)

            # ---- tail: nq2/3 finish with their sk15 matmul + normalize;
            # nq0/1 via the staged partial + one matmul + adds; the fused
            # y chunks run on the PE while the DVE normalizes ahead.
            def av2b_norm(nq):
                av2b = psum.tile([P, 512], F32, tag="av", bufs=2, name="ps_a2b")
                nc.tensor.matmul(
                    av2b,
                    lhsT=vp[2][:, ts(NSK - 1, P)],
                    rhs=pts2[NSK - 1][:, ts(nq, 512)],
                    start=True,
                    stop=True,
                )
                # half-adds keep every operand pair partition-aligned (the
                # BIR verifier rejects SBUF+SBUF inputs at different bases)
                s2l = rlpool.tile([D, 512], F32, name="s2l", tag="s2l", bufs=2)
                nc.vector.tensor_add(out=s2l, in0=av2b[0:D, :], in1=a2sb[nq][0:D, :])
                s2v = rlpool.tile([D, 512], F32, name="s2v", tag="s2v", bufs=2)
                nc.vector.tensor_add(out=s2v, in0=av2b[D:P, :], in1=a2sb[nq][D:P, :])
                rr2 = rlpool.tile([D, 512], F32, name="rr2", tag="rr")
                nc.vector.reciprocal_approx_fast(out=rr2, in_=s2l)
                nc.vector.tensor_mul(out=outT2[0:D, ts(nq, 512)], in0=s2v, in1=rr2)

            av23[0].emit(NSK)       # nq2 sk15 + norm
            av23[1].emit(NSK)       # nq3 sk15 + norm
            av2b_norm(0)
            y_chunk(2)
            av2b_norm(1)
            y_chunk(3)
            y_chunk(0)
            y_chunk(1)
    return nc


_CACHE = threading.Lock(), {}


def _get_nc():
    lock, cache = _CACHE
    with lock:
        if "nc" not in cache:
            nc = bacc.Bacc("TRN2", target_bir_lowering=False, debug=False)
            _build_kernel(nc)
            nc.compile()
            cache["nc"] = nc
        return cache["nc"]


def _shard_inputs(x, w_qkv, b_qkv, w_proj):
    """Build the 8 per-core input maps (host-side sharding/layout)."""
    qscale = float(JSCALE) / 8.0   # fold the Schraudolph J scale into q
    in_maps = []
    for c in range(NCORES):
        b = c // 4
        hg = c % 4
        h0 = HPC * hg
        qc = [np.s_[D * (h0 + i) : D * (h0 + i + 1)] for i in range(HPC)]
        kc = [np.s_[EMBED + D * (h0 + i) : EMBED + D * (h0 + i + 1)] for i in range(HPC)]
        vc = [np.s_[2 * EMBED + D * (h0 + i) : 2 * EMBED + D * (h0 + i + 1)] for i in range(HPC)]

        # projected w tiles: [q0|q1], [k0|k1], [q2|k2]; q columns prescaled
        w_qk = np.zeros((EMBED, 3 * P), dtype=np.float32)
        halves = [
            (0, 0, qc[0], qscale), (0, 1, qc[1], qscale),
            (1, 0, kc[0], 1.0), (1, 1, kc[1], 1.0),
            (2, 0, qc[2], qscale), (2, 1, kc[2], 1.0),
        ]
        for t, half, cols, sc in halves:
            w_qk[:, t * P + half * D : t * P + half * D + D] = w_qkv[:, cols] * sc
        # q biases only: col 0 = [q0|q1], col 1 rows 64:128 = q2. The k
        # biases are constant over keys and cancel in the softmax; the v
        # bias is applied on the host.
        b2 = np.zeros((P, 2), dtype=np.float32)
        b2[0:D, 0] = b_qkv[qc[0]] * qscale
        b2[D:P, 0] = b_qkv[qc[1]] * qscale
        b2[D:P, 1] = b_qkv[qc[2]] * qscale

        w_v = np.concatenate([w_qkv[:, s] for s in vc], axis=1)
        # w_proj rows for these heads; B half zero-padded to K=128
        w_p = np.zeros((2 * P, EMBED), dtype=np.float32)
        w_p[0:P] = w_proj[D * h0 : D * h0 + P]
        w_p[P : P + D] = w_proj[D * h0 + P : D * (h0 + HPC)]
        in_maps.append(
            {
                "xT": np.ascontiguousarray(x[b].T).astype(np.float16),
                "w_qk": w_qk.astype(np.float16),
                "b2": b2,
                "w_v": np.ascontiguousarray(w_v).astype(np.float16),
                "w_p": w_p.astype(np.float16),
            }
        )
    return in_maps


def kernel(x, w_qkv, b_qkv, w_proj, b_proj, _results_hook=None):
    x = np.asarray(x, dtype=np.float32)
    w_qkv = np.asarray(w_qkv, dtype=np.float32)
    b_qkv = np.asarray(b_qkv, dtype=np.float32)
    w_proj = np.asarray(w_proj, dtype=np.float32)
    b_proj = np.asarray(b_proj, dtype=np.float32)

    nc = _get_nc()
    in_maps = _shard_inputs(x, w_qkv, b_qkv, w_proj)
    res = run_bass_kernel_spmd(nc, in_maps, core_ids=list(range(NCORES)))
    if _results_hook is not None:
        _results_hook(res)

    # unshard: sum the 4 head-group partials per batch, add bias terms
    b_v = b_qkv[2 * EMBED :]
    bias_row = b_v @ w_proj + b_proj  # [768]
    out = np.empty((B, S, EMBED), dtype=np.float32)
    for b in range(B):
        acc = np.zeros((EMBED, S), dtype=np.float32)
        for hg in range(4):
            acc += res.results[4 * b + hg]["y"].astype(np.float32)
        out[b] = acc.T + bias_row
    return out
